# revision 21
# baseline (speedup 1.0000x reference)
"""Trainium2 Bass kernel for nn_Block_72146860638363 (Performer/FAVOR+ block).

Sharding: data-parallel over batch B=8, one batch element per NeuronCore,
no collectives. All params + SPE codes replicated per core.

v2: fp8 DoubleRow matmuls for the q/k projection, SPE filter and output
projection (numerically safe: FAVOR+ normalization cancels the quantization
error); merged double-LN; token-major fc2 (no output transposes); pipelined
stage boundaries; head loop software-pipelined (SPE of head h+1 ahead of
FAVOR of head h); PSUM->SBUF copies balanced across DVE/ACT.
"""
import json
import math
import sys

for _p in ("/opt/trn_rl_repo", "/root/.axon_site/_ro/trn_rl_repo"):
    if _p not in sys.path:
        sys.path.insert(0, _p)

import numpy as np
import ml_dtypes

import concourse.bass as bass
import concourse.mybir as mybir
import concourse.tile as tile
from concourse.masks import make_identity

BF16NP = ml_dtypes.bfloat16
F32 = mybir.dt.float32
BF16 = mybir.dt.bfloat16
FP8 = mybir.dt.float8e4
E4NP = mybir.dt.np(mybir.dt.float8e4)
AF = mybir.ActivationFunctionType
ALU = mybir.AluOpType
AX = mybir.AxisListType
DR = mybir.MatmulPerfMode.DoubleRow

B, N, DIM, H, D = 8, 1024, 1536, 12, 128
M, R, S = 64, 64, 5
P = 128
NT = N // P          # 8 token tiles
DC = DIM // P        # 12 feature chunks
CP = DC // 2         # 6 contraction pairs (DoubleRow K=256)
FC1 = 4 * DIM        # 6144
OT1 = FC1 // P       # 48
EPS_LN = 1e-5
EPS_K = 1e-4

# fp8 static scales (powers of two; folded out at PSUM copies)
SH = 8.0             # h stored as 8*h in fp8
SW = 64.0            # weights stored as 64*W in fp8
SOM = 8.0            # om table stored as 8*om (bf16)
SZZ = 256.0          # zz codes stored as 256*zz in fp8
SY = 16.0            # yT stored as 16*y in fp8


# ---------------------------------------------------------------------------
# BIR post-processing: this container's walrus rejects >1 sem wait per
# instruction (>2 for EventSemaphore). Split extra waits onto wait-only
# Drain carriers inserted just before, same engine.
def _split_multiwait(bir_json_bytes: bytes) -> bytes:
    m = json.loads(bir_json_bytes)
    changed = False
    for fn in m.get("functions", []):
        for bb in fn.get("blocks", []):
            new_insts = []
            for inst in bb.get("instructions", []):
                si = inst.get("sync_info") or {}
                waits = si.get("on_wait") or []
                cap = 2 if inst.get("opcode") == "EventSemaphore" else 1
                if len(waits) > cap:
                    changed = True
                    for i, w in enumerate(waits[:-cap]):
                        new_insts.append({
                            "debug": inst.get("debug", 0),
                            "engine": inst["engine"],
                            "ins": [],
                            "is_reset_sema": False,
                            "name": f"{inst['name']}_w{i}",
                            "opcode": "Drain",
                            "outs": [],
                            "sync_info": {"on_update": [], "on_wait": [w]},
                        })
                    si["on_wait"] = waits[-cap:]
                new_insts.append(inst)
            bb["instructions"] = new_insts
    return json.dumps(m).encode() if changed else bir_json_bytes


def _patch_nc(nc):
    orig = nc.to_json_bytes
    nc.to_json_bytes = lambda: _split_multiwait(orig())
    return nc


# ---------------------------------------------------------------------------
# Host preprocessing
def _sigmoid(x):
    return 1.0 / (1.0 + np.exp(-x))


def _softplus(x):
    return np.logaddexp(0.0, x)


def _bf(a):
    return np.ascontiguousarray(a).astype(BF16NP)


def _f8(a):
    return np.ascontiguousarray(np.clip(a, -224.0, 224.0)).astype(E4NP)


def host_prep(inputs):
    f32 = np.float32
    g = {k: np.asarray(v) for k, v in inputs.items()}

    shared = {}
    Wqkv = g["Wqkv"].astype(f32)                              # (4608, 1536)
    # q/k weights, DoubleRow layout: wqk8[o, p, c*256 + j*128 + m]
    #   = 64 * Wqkv[o*128+m, 256c+128j+p]
    Wqk = Wqkv[:2 * DIM].reshape(24, P, CP, 2, P)             # [o, m, c, j, p]
    shared["wqk8"] = _f8(SW * Wqk.transpose(0, 4, 2, 3, 1)
                         .reshape(24, P, CP * 2 * P))         # (24, 128, 1536)
    shared["bqkv_pt"] = np.ascontiguousarray(
        g["bqkv"].astype(f32).reshape(36, P).T)               # (128, 36)
    # v weights: token-major matmul rhs, wvT[d*128+p, f] = Wqkv[2D+f, d*128+p]
    shared["wvT"] = _bf(Wqkv[2 * DIM:].T)                     # (1536, 1536)

    Wproj = g["Wproj"].astype(f32)                            # (1536, 1536)
    # proj weights DoubleRow: wpj8[c, p, j*1536+dout] = 64*Wproj[dout, 256c+128j+p]
    Wp = Wproj.reshape(DIM, CP, 2, P)                         # [dout, c, j, p]
    shared["wpj8"] = _f8(SW * Wp.transpose(1, 3, 2, 0)
                         .reshape(CP, P, 2 * DIM))            # (6, 128, 3072)

    # fc1 weights pre-shuffled: w1r[o, p, dc*128+m] = W1T[dc*128+p, o*128+m]
    W1T = g["W1"].astype(f32).T                               # (1536, 6144)
    w1r = W1T.reshape(DC, P, OT1, P).transpose(2, 1, 0, 3)
    shared["w1r"] = _bf(w1r.reshape(OT1, P, DC * P))          # (48, 128, 1536)
    shared["b1_pt"] = np.ascontiguousarray(
        g["b1"].astype(f32).reshape(OT1, P).T)                # (128, 48)
    # fc2 weights token-major blocks: w2b[j, oc, p, n] = W2T[oc*128+p, j*512+n]
    W2T = g["W2"].astype(f32).T                               # (6144, 1536)
    w2b = W2T.reshape(OT1, P, 3, 512).transpose(2, 0, 1, 3)
    shared["w2b"] = _bf(w2b.reshape(3, OT1, P, 512))          # (3, 48, 128, 512)

    shared["wT"] = _bf(g["w"].astype(f32).T)                  # (128, 64)

    # ---- SPE folds (float64 internally, tiny tensors) ----
    f = _sigmoid(g["spe_freqs"].astype(np.float64)) * 0.5     # (H,D,S)
    off = g["spe_offsets"].astype(np.float64)                 # (H,D,S)
    gains = _softplus(g["spe_gains"].astype(np.float64))      # (H,D,S)
    gate = _sigmoid(g["spe_gate"].astype(np.float64))         # (H,D)
    zn = g["z_noise"].astype(np.float64)[0]                   # (H,D,2S,R)
    gn = g["gating_noise"].astype(np.float64)                 # (H,D,R)

    # zz[h,d,2s+c,r] = z_noise * gains[s]
    zz = zn * np.repeat(gains, 2, axis=-1)[..., None]         # (H,D,2S,R)
    # rotate by offsets so the device only needs cos/sin of the raw phase
    co = np.cos(off)[..., None]                               # (H,D,S,1)
    so = np.sin(off)[..., None]
    zzq = np.empty_like(zz)
    zzq[:, :, 0::2, :] = co * zz[:, :, 0::2, :] + so * zz[:, :, 1::2, :]
    zzq[:, :, 1::2, :] = -so * zz[:, :, 0::2, :] + co * zz[:, :, 1::2, :]

    scale = (R * D) ** 0.25
    r4 = R ** 0.25
    dn = float(2 * R) ** (-0.25)
    e_fold = (np.sqrt(1.0 - gate) * (dn / (scale * r4)))[:, :, None, None]
    zzq_dev = zzq * e_fold                                    # (H,D,2S,R)
    zzk_dev = zz * e_fold
    gp_dev = np.sqrt(gate)[:, :, None] * gn * (dn / ((D * R) ** 0.25 * r4))

    # device layouts: zz[h, d, sc*64+r] bf16, sc = 2s+c (c=0 cos, c=1 sin)
    shared["zzq"] = _bf(zzq_dev.reshape(H, D, 2 * S * R))     # (H, 128, 640)
    shared["zzk"] = _bf(zzk_dev.reshape(H, D, 2 * S * R))
    shared["gp"] = _bf(gp_dev)                                # (H, D, R)

    # trig table om[h, s, d, c*N+n], c=0 cos, c=1 sin
    n_idx = np.arange(N, dtype=np.float64)
    om = np.empty((H, S, D, 2 * N), dtype=BF16NP)
    for s in range(S):
        ph = 2.0 * math.pi * f[:, :, s:s + 1] * n_idx[None, None, :]  # (H,D,N)
        om[:, s, :, :N] = np.cos(ph).astype(BF16NP)
        om[:, s, :, N:] = np.sin(ph).astype(BF16NP)
    shared["om"] = om

    # flags for build specialization (all hold for the spec's fills)
    ones = lambda a: bool(np.all(np.asarray(a) == 1.0))
    zeros = lambda a: bool(np.all(np.asarray(a) == 0.0))
    flags = dict(
        trivial_ln=(ones(g["ln0_g"]) and zeros(g["ln0_b"]) and
                    ones(g["lna_g"]) and zeros(g["lna_b"]) and
                    ones(g["ln2_g"]) and zeros(g["ln2_b"])),
        zero_bv=zeros(g["bqkv"][3072:]),
        zero_bproj=zeros(g["bproj"]),
        zero_b2=zeros(g["b2"]),
    )
    if not flags["trivial_ln"]:
        for nm in ("ln0_g", "ln0_b", "lna_g", "lna_b", "ln2_g", "ln2_b"):
            shared[nm] = np.ascontiguousarray(g[nm].astype(f32)[None, :])
    if not flags["zero_bv"]:
        shared["bv_row"] = np.ascontiguousarray(g["bqkv"].astype(f32)[None, 3072:])
    if not flags["zero_bproj"]:
        shared["bproj_row"] = np.ascontiguousarray(g["bproj"].astype(f32)[None, :])
    if not flags["zero_b2"]:
        shared["b2_row"] = np.ascontiguousarray(g["b2"].astype(f32)[None, :])

    x = np.ascontiguousarray(g["x"].astype(f32))              # (B, N, DIM)
    return shared, x, flags


# ---------------------------------------------------------------------------
# Program build
_NC_CACHE = {}

_SHAPES = dict(
    x=((N, DIM), F32),
    wqk8=((24, P, CP * 2 * P), FP8),
    bqkv_pt=((P, 36), F32),
    wvT=((DIM, DIM), BF16),
    wpj8=((CP, P, 2 * DIM), FP8),
    w1r=((OT1, P, DC * P), BF16),
    b1_pt=((P, OT1), F32),
    w2b=((3, OT1, P, 512), BF16),
    wT=((P, M), BF16),
    zzq=((H, D, 2 * S * R), BF16),
    zzk=((H, D, 2 * S * R), BF16),
    gp=((H, D, R), BF16),
    om=((H, S, D, 2 * N), BF16),
    ln0_g=((1, DIM), F32), ln0_b=((1, DIM), F32),
    lna_g=((1, DIM), F32), lna_b=((1, DIM), F32),
    ln2_g=((1, DIM), F32), ln2_b=((1, DIM), F32),
    bv_row=((1, DIM), F32),
    bproj_row=((1, DIM), F32),
    b2_row=((1, DIM), F32),
)


def _input_names(flags):
    names = ["x", "wqk8", "bqkv_pt", "wvT", "wpj8", "w1r", "b1_pt", "w2b",
             "wT", "zzq", "zzk", "gp", "om"]
    if not flags["trivial_ln"]:
        names += ["ln0_g", "ln0_b", "lna_g", "lna_b", "ln2_g", "ln2_b"]
    if not flags["zero_bv"]:
        names.append("bv_row")
    if not flags["zero_bproj"]:
        names.append("bproj_row")
    if not flags["zero_b2"]:
        names.append("b2_row")
    return names


def build_nc(flags, dbg=(), trace_sim=False):
    nc = bass.Bass("TRN2", debug=False)
    ins = {}
    for name in _input_names(flags):
        shp, dt = _SHAPES[name]
        ins[name] = nc.dram_tensor(name, shp, dt, kind="ExternalInput").ap()
    outs = {"out": nc.dram_tensor("out", (N, DIM), F32, kind="ExternalOutput").ap()}
    for d in dbg:
        shp, dt = _DBG_SHAPES[d]
        outs[d] = nc.dram_tensor(d, shp, dt, kind="ExternalOutput").ap()
    with tile.TileContext(nc, trace_sim=trace_sim) as tc:
        emit(tc, outs, ins, flags, dbg=set(dbg))
    return _patch_nc(nc)


_DBG_SHAPES = dict(
    d_h=((N, DIM), BF16),          # post double-LN, token major
    d_q0=((P, N), BF16),           # q head0 feature-major
    d_v=((N, DIM), BF16),          # v token-major
    d_qh0=((P, N), BF16),          # qh~ head0 feature-major
    d_yt0=((P, N), FP8),           # yT head0 scaled by SY
    d_r1=((N, DIM), F32),
    d_h2=((N, DIM), BF16),
    d_a10=((P, N), BF16),          # fc1 act o-tile 0, feature-major
)


# ---------------------------------------------------------------------------
# Runner
def _run(nc, in_maps):
    from concourse import bass_utils
    return bass_utils.run_bass_kernel_spmd(nc, in_maps, core_ids=list(range(B)))


def get_nc(flags, dbg=()):
    key = (tuple(sorted(flags.items())), tuple(sorted(dbg)))
    if key not in _NC_CACHE:
        _NC_CACHE[key] = build_nc(flags, dbg)
    return _NC_CACHE[key]


def kernel(**inputs):
    shared, x, flags = host_prep(inputs)
    nc = get_nc(flags)
    in_maps = [dict(shared, x=np.ascontiguousarray(x[c])) for c in range(B)]
    res = _run(nc, in_maps)
    out = np.stack([res.results[c]["out"] for c in range(B)], axis=0)
    return out.astype(np.float32)


# ===========================================================================
# The device program
# ===========================================================================
def _ln_pass(nc, sp, in_tile, out_tile, eps_t, gb=None):
    """Single LayerNorm on one (P, DIM) tile: bn_stats + ACT apply."""
    st6 = sp.tile([P, 3 * 6], F32, tag="ln_st6", name="ln_st6")
    for gi in range(3):
        nc.vector.bn_stats(st6[:, gi * 6:(gi + 1) * 6],
                           in_tile[:, gi * 512:(gi + 1) * 512])
    mv = sp.tile([P, 2], F32, tag="ln_mv", name="ln_mv")
    nc.vector.bn_aggr(mv[:], st6[:].rearrange("p (g s) -> p g s", s=6))
    sd = sp.tile([P, 1], F32, tag="ln_sd", name="ln_sd")
    nc.scalar.activation(sd[:], mv[:, 1:2], AF.Sqrt, bias=eps_t[:], scale=1.0)
    rstd = sp.tile([P, 1], F32, tag="ln_rstd", name="ln_rstd")
    nc.vector.reciprocal(rstd[:], sd[:])
    nbias = sp.tile([P, 1], F32, tag="ln_nb", name="ln_nb")
    nc.vector.scalar_tensor_tensor(nbias[:], in0=mv[:, 0:1], scalar=-1.0,
                                   in1=rstd[:], op0=ALU.mult, op1=ALU.mult)
    if gb is None:
        nc.scalar.activation(out_tile[:], in_tile[:], AF.Identity,
                             bias=nbias[:], scale=rstd[:])
    else:
        g_b, b_b = gb
        tmp = sp.tile([P, DIM], F32, tag="ln_tmp", name="ln_tmp")
        nc.scalar.activation(tmp[:], in_tile[:], AF.Identity,
                             bias=nbias[:], scale=rstd[:])
        nc.vector.tensor_tensor(tmp[:], tmp[:], g_b[:], ALU.mult)
        nc.vector.tensor_tensor(out_tile[:], tmp[:], b_b[:], ALU.add)


def _ln_double(nc, sp, in_tile, out_tile, eps_t):
    """Merged LN(LN(x)) for trivial g/b: one stats pass, combined scale.

    y = (x-mu)*r1, r1 = rsqrt(var+eps); mean(y)=0, var(y)=var*r1^2;
    h = y * rsqrt(var*r1^2 + eps) = x*r + (-mu*r), r = r1*r2.
    """
    st6 = sp.tile([P, 3 * 6], F32, tag="ln_st6", name="ln_st6")
    for gi in range(3):
        nc.vector.bn_stats(st6[:, gi * 6:(gi + 1) * 6],
                           in_tile[:, gi * 512:(gi + 1) * 512])
    mv = sp.tile([P, 2], F32, tag="ln_mv", name="ln_mv")
    nc.vector.bn_aggr(mv[:], st6[:].rearrange("p (g s) -> p g s", s=6))
    sd = sp.tile([P, 1], F32, tag="ln_sd", name="ln_sd")
    nc.scalar.activation(sd[:], mv[:, 1:2], AF.Sqrt, bias=eps_t[:], scale=1.0)
    r1 = sp.tile([P, 1], F32, tag="ln_r1", name="ln_r1")
    nc.vector.reciprocal(r1[:], sd[:])
    vy = sp.tile([P, 1], F32, tag="ln_vy", name="ln_vy")
    nc.vector.tensor_tensor(vy[:], mv[:, 1:2], r1[:], ALU.mult)
    nc.vector.tensor_tensor(vy[:], vy[:], r1[:], ALU.mult)
    sd2 = sp.tile([P, 1], F32, tag="ln_sd2", name="ln_sd2")
    nc.scalar.activation(sd2[:], vy[:], AF.Sqrt, bias=eps_t[:], scale=1.0)
    r2 = sp.tile([P, 1], F32, tag="ln_r2", name="ln_r2")
    nc.vector.reciprocal(r2[:], sd2[:])
    r = sp.tile([P, 1], F32, tag="ln_r", name="ln_r")
    nc.vector.tensor_tensor(r[:], r1[:], r2[:], ALU.mult)
    nbias = sp.tile([P, 1], F32, tag="ln_nb", name="ln_nb")
    nc.vector.scalar_tensor_tensor(nbias[:], in0=mv[:, 0:1], scalar=-1.0,
                                   in1=r[:], op0=ALU.mult, op1=ALU.mult)
    nc.scalar.activation(out_tile[:], in_tile[:], AF.Identity,
                         bias=nbias[:], scale=r[:])


def emit(tc, outs, ins, flags, dbg=()):
    from contextlib import ExitStack
    nc = tc.nc
    trivial_ln = flags["trivial_ln"]

    with ExitStack() as ctx:
        const = ctx.enter_context(tc.tile_pool(name="const", bufs=1))
        sp = ctx.enter_context(tc.tile_pool(name="smalls", bufs=4))
        rp = ctx.enter_context(tc.tile_pool(name="rp", bufs=1))

        eye_bf = const.tile([P, P], BF16, tag="eye_bf", name="eye_bf")
        make_identity(nc, eye_bf[:])
        eye_f = const.tile([P, P], F32, tag="eye_f", name="eye_f")
        make_identity(nc, eye_f[:])
        ones_col = const.tile([P, 1], BF16, tag="ones_col", name="ones_col")
        nc.vector.memset(ones_col[:], 1.0)
        eps_t = const.tile([P, 1], F32, tag="eps_t", name="eps_t")
        nc.vector.memset(eps_t[:], EPS_LN)
        ones_row = const.tile([1, P], F32, tag="ones_row", name="ones_row")
        nc.vector.memset(ones_row[:], 1.0)
        epsk_t = const.tile([P, 1], F32, tag="epsk_t", name="epsk_t")
        nc.vector.memset(epsk_t[:], EPS_K)
        bqkv_pt = const.tile([P, 36], F32, tag="bqkv_pt", name="bqkv_pt")
        nc.sync.dma_start(bqkv_pt[:], ins["bqkv_pt"])
        b1_pt = const.tile([P, OT1], F32, tag="b1_pt", name="b1_pt")
        nc.sync.dma_start(b1_pt[:], ins["b1_pt"])
        wT_sb = const.tile([P, M], BF16, tag="wT", name="wT")
        nc.sync.dma_start(wT_sb[:], ins["wT"])

        def bcast_row(name, tag):
            row = const.tile([1, DIM], F32, tag=tag + "_r")
            nc.sync.dma_start(row[:], ins[name])
            t = const.tile([P, DIM], F32, tag=tag)
            nc.gpsimd.partition_broadcast(t[:], row[:])
            return t

        gb0 = gba = gb2 = None
        if not trivial_ln:
            gb0 = (bcast_row("ln0_g", "g0"), bcast_row("ln0_b", "b0"))
            gba = (bcast_row("lna_g", "ga"), bcast_row("lna_b", "ba"))
            gb2 = (bcast_row("ln2_g", "g2"), bcast_row("ln2_b", "b2"))
        bv_b = None if flags["zero_bv"] else bcast_row("bv_row", "bv")
        bproj_b = None if flags["zero_bproj"] else bcast_row("bproj_row", "bpj")
        b2_b = None if flags["zero_b2"] else bcast_row("b2_row", "b2v")

        # x tiles double as the residual stream r1 (in-place updates)
        x_tiles = []
        for t in range(NT):
            xt = rp.tile([P, DIM], F32, tag=f"x{t}", name=f"x{t}")
            nc.sync.dma_start(xt[:], ins["x"][t * P:(t + 1) * P, :])
            x_tiles.append(xt)
        r1 = x_tiles

        # yT8 pool spans attention + proj
        yT8p_cm = tc.tile_pool(name="yT8p", bufs=1, side="right")
        yT8p = yT8p_cm.__enter__()
        yT8 = yT8p.tile([P, H * N], FP8, tag="yT8", name="yT8")

        # pools that live from stage B through D
        with tc.tile_pool(name="qkT", bufs=1) as qkT, \
             tc.tile_pool(name="vtp", bufs=1) as vtp:
            qT_t = [qkT.tile([P, N], BF16, tag=f"qT{hh}", name=f"qT{hh}") for hh in range(H)]
            kT_t = [qkT.tile([P, N], BF16, tag=f"kT{hh}", name=f"kT{hh}") for hh in range(H)]
            v_tok = [vtp.tile([P, DIM], BF16, tag=f"v{t}", name=f"v{t}") for t in range(NT)]
            vsum_row = vtp.tile([1, DIM], F32, tag="vsum", name="vsum")

            # ------------ Stage A+B: LN, transpose, qkv (hf-pipelined) ----
            with tc.tile_pool(name="htp", bufs=1) as htp, \
                 tc.tile_pool(name="lnp", bufs=2) as lnp, \
                 tc.tile_pool(name="wqkp", bufs=3) as wqkp, \
                 tc.tile_pool(name="psB", bufs=2, space="PSUM") as psB, \
                 tc.tile_pool(name="psT", bufs=4, space="PSUM") as psT:
                hT = [htp.tile([P, N], BF16, tag=f"hT{d}", name=f"hT{d}")
                      for d in range(DC)]
                # single fp8 transposed-h tile: hT8[p, d*N + n] = SH * hT
                hT8 = htp.tile([P, DC * N], FP8, tag="h8x", name="h8x")

                # interleave q/k o-tiles so head h is ready after 2 tiles
                o_order = []
                for hh in range(H):
                    o_order += [hh, H + hh]

                for hf in range(2):
                    for t in range(4 * hf, 4 * hf + 4):
                        htok = lnp.tile([P, DIM], BF16, tag="ln_h", name="ln_h")
                        if trivial_ln:
                            _ln_double(nc, lnp, x_tiles[t], htok, eps_t)
                        else:
                            y = lnp.tile([P, DIM], BF16, tag="ln_y", name="ln_y")
                            _ln_pass(nc, lnp, x_tiles[t], y, eps_t, gb=gb0)
                            _ln_pass(nc, lnp, y, htok, eps_t, gb=gba)
                        if "d_h" in dbg:
                            nc.sync.dma_start(
                                outs["d_h"][t * P:(t + 1) * P, :], htok[:])
                        for d in range(DC):
                            tps = psT.tile([P, P], BF16, tag="tps", name="tps")
                            nc.tensor.transpose(tps[:], htok[:, d * P:(d + 1) * P],
                                                eye_bf[:])
                            if d % 2 == 0:
                                nc.vector.tensor_copy(
                                    hT[d][:, t * P:(t + 1) * P], tps[:])
                            else:
                                nc.scalar.activation(
                                    hT[d][:, t * P:(t + 1) * P], tps[:],
                                    AF.Copy, scale=1.0)
                    # fp8 copies of this hf half (scaled by SH)
                    for d in range(DC):
                        nc.vector.tensor_scalar(
                            out=hT8[:, d * N + hf * 512: d * N + hf * 512 + 512],
                            in0=hT[d][:, hf * 512:(hf + 1) * 512],
                            scalar1=SH, scalar2=None, op0=ALU.mult)
                    # q/k matmuls for this hf half
                    hT8_3d = hT8[:].rearrange("p (d n) -> p d n", d=DC)
                    for o in o_order:
                        wt = wqkp.tile([P, CP * 2 * P], FP8, tag="wqk",
                                       name="wqk")
                        nc.sync.dma_start(wt[:], ins["wqk8"][o])
                        wt3 = wt[:].rearrange("p (c j m) -> p c j m", c=CP, j=2)
                        dest = qT_t[o] if o < H else kT_t[o - H]
                        pst = psB.tile([P, 512], F32, tag="psB", name="psB")
                        for c in range(CP):
                            nc.tensor.matmul(
                                pst[:], wt3[:, c],
                                hT8_3d[:, 2 * c:2 * c + 2,
                                       hf * 512:(hf + 1) * 512],
                                start=(c == 0), stop=(c == CP - 1),
                                perf_mode=DR)
                        nc.scalar.activation(
                            dest[:, hf * 512:(hf + 1) * 512], pst[:],
                            AF.Identity, bias=bqkv_pt[:, o:o + 1],
                            scale=1.0 / (SH * SW))

                # v part: token-major, bf16
                with tc.tile_pool(name="wvp", bufs=1) as wvp:
                    for oc in range(3):
                        wv = []
                        for d in range(DC):
                            wvt = wvp.tile([P, 512], BF16, tag=f"wv{d}",
                                           name=f"wv{d}", bufs=1)
                            nc.sync.dma_start(
                                wvt[:],
                                ins["wvT"][d * P:(d + 1) * P,
                                           oc * 512:(oc + 1) * 512])
                            wv.append(wvt)
                        for t in range(NT):
                            pst = psB.tile([P, 512], F32, tag="psB", name="psB")
                            for d in range(DC):
                                nc.tensor.matmul(
                                    pst[:], hT[d][:, t * P:(t + 1) * P],
                                    wv[d][:],
                                    start=(d == 0), stop=(d == DC - 1))
                            nc.scalar.activation(
                                v_tok[t][:, oc * 512:(oc + 1) * 512], pst[:],
                                AF.Copy, scale=1.0)
                            if bv_b is not None:
                                nc.vector.tensor_tensor(
                                    v_tok[t][:, oc * 512:(oc + 1) * 512],
                                    v_tok[t][:, oc * 512:(oc + 1) * 512],
                                    bv_b[:, oc * 512:(oc + 1) * 512],
                                    ALU.add)

                # residual init: r1 = x + vf, in place
                for t in range(NT):
                    nc.vector.tensor_tensor(r1[t][:], x_tiles[t][:],
                                            v_tok[t][:], ALU.add)
                # vsum_row[0, f] = sum_n v[n, f] (for the ctx eps-correction)
                for oc in range(3):
                    vsps_ = psB.tile([1, 512], F32, tag="psB1", name="psB1",
                                     bufs=2)
                    for t in range(NT):
                        nc.tensor.matmul(vsps_[:], ones_col[:],
                                         v_tok[t][:, oc * 512:(oc + 1) * 512],
                                         start=(t == 0), stop=(t == NT - 1))
                    nc.vector.tensor_copy(vsum_row[:, oc * 512:(oc + 1) * 512],
                                          vsps_[:])

            if "d_q0" in dbg:
                nc.sync.dma_start(outs["d_q0"], qT_t[0][:])
            if "d_v" in dbg:
                for t in range(NT):
                    nc.sync.dma_start(outs["d_v"][t * P:(t + 1) * P, :],
                                      v_tok[t][:])

            # ------------ Stage C+D: attention, head-pipelined ------------
            with tc.tile_pool(name="attn", bufs=2) as ap, \
                 tc.tile_pool(name="attn1", bufs=1) as ap1, \
                 tc.tile_pool(name="omp", bufs=2) as omp, \
                 tc.tile_pool(name="psSPE", bufs=4, space="PSUM") as psS, \
                 tc.tile_pool(name="psSm", bufs=2, space="PSUM") as psm, \
                 tc.tile_pool(name="psQ", bufs=1, space="PSUM") as psq:
                pools = dict(
                    ap=ap, ap1=ap1, omp=omp, psS=psS, psm=psm, psq=psq, sp=sp,
                    eye_bf=eye_bf, eye_f=eye_f, ones_col=ones_col,
                    ones_row=ones_row, wT_sb=wT_sb, epsk_t=epsk_t,
                    vsum_row=vsum_row)
                qkh = {}
                qkh[0] = _emit_spe(tc, nc, pools, ins, outs, dbg, 0,
                                   qT_t[0], kT_t[0])
                for hh in range(H):
                    if hh + 1 < H:
                        qkh[hh + 1] = _emit_spe(tc, nc, pools, ins, outs, dbg,
                                                hh + 1, qT_t[hh + 1],
                                                kT_t[hh + 1])
                    qhT, khT = qkh.pop(hh)
                    _emit_favor(tc, nc, pools, ins, outs, dbg, hh,
                                qhT, khT, v_tok, yT8)

        # -------- Stage E: proj + residual + LN2 (t-pipelined) ------------
        h2T_cm = tc.tile_pool(name="h2tp", bufs=1)
        h2tp = h2T_cm.__enter__()
        h2T = [h2tp.tile([P, N], BF16, tag=f"h2T{d}", name=f"h2T{d}")
               for d in range(DC)]
        with tc.tile_pool(name="wpp", bufs=1) as wpp, \
             tc.tile_pool(name="ln2p", bufs=2) as ln2p, \
             tc.tile_pool(name="psE", bufs=3, space="PSUM") as psE, \
             tc.tile_pool(name="psT2", bufs=4, space="PSUM") as psT2:
            wp = []
            for c in range(CP):
                wpt = wpp.tile([P, 2 * DIM], FP8, tag=f"wp{c}",
                               name=f"wp{c}")
                nc.sync.dma_start(wpt[:], ins["wpj8"][c])
                wp.append(wpt)
            yT8_3d = yT8[:].rearrange("p (d n) -> p d n", d=H)
            for t in range(NT):
                for oc in range(3):
                    pst = psE.tile([P, 512], F32, tag="psE", name="psE")
                    for c in range(CP):
                        nc.tensor.matmul(
                            pst[:],
                            yT8_3d[:, 2 * c:2 * c + 2, t * P:(t + 1) * P],
                            wp[c][:].rearrange("p (j f) -> p j f", j=2)
                            [:, :, oc * 512:(oc + 1) * 512],
                            start=(c == 0), stop=(c == CP - 1),
                            perf_mode=DR)
                    sl = r1[t][:, oc * 512:(oc + 1) * 512]
                    nc.vector.scalar_tensor_tensor(
                        sl, in0=pst[:], scalar=1.0 / (SY * SW),
                        in1=sl, op0=ALU.mult, op1=ALU.add)
                    if bproj_b is not None:
                        nc.vector.tensor_tensor(
                            sl, sl, bproj_b[:, oc * 512:(oc + 1) * 512],
                            ALU.add)
                if "d_r1" in dbg:
                    nc.sync.dma_start(
                        outs["d_r1"][t * P:(t + 1) * P, :], r1[t][:])
                # LN2 + transposes for this tile
                h2 = ln2p.tile([P, DIM], BF16, tag="h2", name="h2")
                _ln_pass(nc, ln2p, r1[t], h2, eps_t,
                         gb=(None if trivial_ln else gb2))
                if "d_h2" in dbg:
                    nc.sync.dma_start(
                        outs["d_h2"][t * P:(t + 1) * P, :], h2[:])
                for d in range(DC):
                    tps = psT2.tile([P, P], BF16, tag="tps2", name="tps2")
                    nc.tensor.transpose(tps[:], h2[:, d * P:(d + 1) * P],
                                        eye_bf[:])
                    if d % 2 == 0:
                        nc.vector.tensor_copy(
                            h2T[d][:, t * P:(t + 1) * P], tps[:])
                    else:
                        nc.scalar.activation(
                            h2T[d][:, t * P:(t + 1) * P], tps[:],
                            AF.Copy, scale=1.0)
        yT8p_cm.__exit__(None, None, None)

        # ---------------- Stage F: MLP ------------------------------------
        _emit_mlp(tc, nc, ctx, ins, outs, dbg, flags, r1, h2T, h2T_cm,
                  b1_pt, b2_b, sp)


def _emit_spe(tc, nc, pools, ins, outs, dbg, hh, qT_h, kT_h):
    """SPE filter for one head: returns (qhT, khT) bf16 tiles (P, N)."""
    ap, omp, psS, sp = pools["ap"], pools["omp"], pools["psS"], pools["sp"]

    zzq_t = ap.tile([P, 2 * S * R], BF16, tag="zzq", name="zzq", bufs=2)
    nc.gpsimd.dma_start(zzq_t[:], ins["zzq"][hh])
    zzk_t = ap.tile([P, 2 * S * R], BF16, tag="zzk", name="zzk", bufs=2)
    nc.gpsimd.dma_start(zzk_t[:], ins["zzk"][hh])
    gp_t = ap.tile([P, R], BF16, tag="gp", name="gp", bufs=2)
    nc.gpsimd.dma_start(gp_t[:], ins["gp"][hh])

    qhT = ap.tile([P, N], BF16, tag="qhT", name="qhT")
    khT = ap.tile([P, N], BF16, tag="khT", name="khT")
    ps_q = [psS.tile([M, 512], F32, tag="spe", name="spe") for _ in range(2)]
    ps_k = [psS.tile([M, 512], F32, tag="spe", name="spe") for _ in range(2)]
    for s in range(S):
        om_t = omp.tile([P, 2 * N], BF16, tag="om", name="om", bufs=4)
        nc.gpsimd.dma_start(om_t[:], ins["om"][hh, s])
        Gq = omp.tile([P, 2 * N], BF16, tag="Gq", name="Gq")
        Gk = omp.tile([P, 2 * N], BF16, tag="Gk", name="Gk")
        for c in range(2):
            nc.vector.tensor_tensor(Gq[:, c * N:(c + 1) * N], qT_h[:],
                                    om_t[:, c * N:(c + 1) * N], ALU.mult)
            nc.vector.tensor_tensor(Gk[:, c * N:(c + 1) * N], kT_h[:],
                                    om_t[:, c * N:(c + 1) * N], ALU.mult)
        for c in range(2):
            sc = 2 * s + c
            for hf in range(2):
                nc.tensor.matmul(ps_q[hf][:],
                                 zzq_t[:, sc * R:(sc + 1) * R],
                                 Gq[:, c * N + hf * 512:c * N + hf * 512 + 512],
                                 start=(sc == 0), stop=(sc == 2 * S - 1))
                nc.tensor.matmul(ps_k[hf][:],
                                 zzk_t[:, sc * R:(sc + 1) * R],
                                 Gk[:, c * N + hf * 512:c * N + hf * 512 + 512],
                                 start=(sc == 0), stop=(sc == 2 * S - 1))
    for hf in range(2):
        nc.scalar.activation(qhT[0:M, hf * 512:(hf + 1) * 512], ps_q[hf][:],
                             AF.Copy, scale=1.0)
        nc.scalar.activation(khT[0:M, hf * 512:(hf + 1) * 512], ps_k[hf][:],
                             AF.Copy, scale=1.0)
        pg = psS.tile([M, 512], F32, tag="spe", name="spe")
        nc.tensor.matmul(pg[:], gp_t[:], qT_h[:, hf * 512:(hf + 1) * 512],
                         start=True, stop=True)
        nc.vector.tensor_copy(qhT[M:P, hf * 512:(hf + 1) * 512], pg[:])
        pg2 = psS.tile([M, 512], F32, tag="spe", name="spe")
        nc.tensor.matmul(pg2[:], gp_t[:], kT_h[:, hf * 512:(hf + 1) * 512],
                         start=True, stop=True)
        nc.vector.tensor_copy(khT[M:P, hf * 512:(hf + 1) * 512], pg2[:])
    if hh == 0 and "d_qh0" in dbg:
        nc.sync.dma_start(outs["d_qh0"], qhT[:])
    return qhT, khT


def _emit_favor(tc, nc, pools, ins, outs, dbg, hh, qhT, khT, v_tok, yT8):
    ap, ap1, psm, psq, sp = (pools["ap"], pools["ap1"], pools["psm"],
                             pools["psq"], pools["sp"])
    eye_bf, eye_f = pools["eye_bf"], pools["eye_f"]
    ones_col, wT_sb = pools["ones_col"], pools["wT_sb"]
    ones_row, epsk_t = pools["ones_row"], pools["epsk_t"]
    vsum_row = pools["vsum_row"]

    def bc(dst, src_row, w):
        bps = psm.tile([P, w], F32, tag="sps", name="bcps")
        nc.tensor.matmul(bps[:], ones_row[:], src_row, start=True, stop=True)
        nc.scalar.activation(dst, bps[:], AF.Copy, scale=1.0)

    # --- k-side: dd_k, diag_k, global max
    dd_k = ap1.tile([P, NT * M], F32, tag="ddk", name="ddk")
    dgk8 = ap.tile([P, NT], F32, tag="dgk8", name="dgk8")
    mxk = ap.tile([P, 1], F32, tag="mxk", name="mxk")
    for t in range(NT):
        pt = psq.tile([P, M + P], F32, tag=f"pq{t % 2}", name="pq")
        nc.tensor.matmul(pt[:, 0:M], khT[:, t * P:(t + 1) * P], wT_sb[:],
                         start=True, stop=True, skip_group_check=True)
        nc.tensor.matmul(pt[:, M:M + P], khT[:, t * P:(t + 1) * P],
                         khT[:, t * P:(t + 1) * P], start=True, stop=True,
                         skip_group_check=True)
        nc.scalar.activation(dd_k[:, t * M:(t + 1) * M], pt[:, 0:M], AF.Copy,
                             scale=1.0)
        scr = sp.tile([P, P], BF16, tag="scr128", name="scr128")
        nc.vector.scalar_tensor_tensor(
            scr[:], in0=pt[:, M:M + P], scalar=0.5, in1=eye_f[:],
            op0=ALU.mult, op1=ALU.mult, accum_out=dgk8[:, t:t + 1])
    nc.vector.tensor_reduce(mxk[:], dd_k[:], AX.X, ALU.max)
    # global max over partitions: PE transpose (128,1)->(1,128), DVE max
    mps = psm.tile([1, P], F32, tag="sps", name="sps")
    nc.tensor.transpose(mps[:], mxk[:], eye_f[:])
    mxrow = sp.tile([1, P], F32, tag="mxrow", name="mxrow")
    nc.vector.tensor_copy(mxrow[:], mps[:])
    mxk1 = sp.tile([1, 1], F32, tag="mxk1", name="mxk1")
    nc.vector.tensor_reduce(mxk1[:], mxrow[:], AX.X, ALU.max)
    mxkb = ap.tile([P, 1], F32, tag="mxkb", name="mxkb")
    bc(mxkb[:], mxk1[:], 1)

    # biask8 = -diag_k - mxk (batched); kp = exp(dd + biask) per tile
    biask8 = ap.tile([P, NT], F32, tag="biask8", name="biask8")
    nc.vector.tensor_scalar(out=biask8[:], in0=dgk8[:], scalar1=-1.0,
                            scalar2=mxkb[:], op0=ALU.mult, op1=ALU.subtract)
    kp = [ap1.tile([P, M], BF16, tag=f"kp{t}", name=f"kp{t}") for t in range(NT)]
    for t in range(NT):
        nc.scalar.activation(kp[t][:], dd_k[:, t * M:(t + 1) * M], AF.Exp,
                             bias=biask8[:, t:t + 1], scale=1.0)

    # ksum (1, M): kp holds exp only, true ksum = ksum_exp + N*eps
    ksps = psm.tile([1, M], F32, tag="sps", name="sps")
    for t in range(NT):
        nc.tensor.matmul(ksps[:], ones_col[:], kp[t][:], start=(t == 0),
                         stop=(t == NT - 1))
    ksrow = sp.tile([1, M], F32, tag="ksrow", name="ksrow")
    nc.vector.tensor_scalar(out=ksrow[:], in0=ksps[:],
                            scalar1=float(N) * EPS_K, scalar2=None,
                            op0=ALU.add)
    ksum_b = ap.tile([P, M], F32, tag="ksum_b", name="ksum_b")
    bc(ksum_b[:], ksrow[:], M)
    kss = sp.tile([1, 1], F32, tag="kss", name="kss")
    nc.vector.tensor_reduce(kss[:], ksrow[:], AX.X, ALU.add)
    seps_r = sp.tile([1, 1], F32, tag="seps_r", name="seps_r")
    nc.vector.tensor_scalar(out=seps_r[:], in0=kss[:],
                            scalar1=8.0 * EPS_K, scalar2=None, op0=ALU.mult)
    seps8 = ap.tile([P, 1], F32, tag="seps8", name="seps8")
    bc(seps8[:], seps_r[:], 1)

    # ctx (M, P): true ctx = ctx_exp + eps * vsum_h
    cps = psm.tile([M, P], F32, tag="sps", name="sps")
    for t in range(NT):
        nc.tensor.matmul(cps[:], kp[t][:],
                         v_tok[t][:, hh * P:(hh + 1) * P],
                         start=(t == 0), stop=(t == NT - 1))
    ctx_sb = ap.tile([M, P], BF16, tag="ctx_sb", name="ctx_sb")
    vs_b = ap.tile([M, P], F32, tag="vs_b", name="vs_b", bufs=2)
    vsps = psm.tile([M, P], F32, tag="sps", name="sps")
    nc.tensor.matmul(vsps[:], ones_row[:, 0:M], vsum_row[:, hh * P:(hh + 1) * P],
                     start=True, stop=True)
    nc.scalar.activation(vs_b[:], vsps[:], AF.Copy, scale=EPS_K)
    nc.vector.tensor_tensor(ctx_sb[:], cps[:], vs_b[:], ALU.add)

    # --- q-side: all 8 tiles' matmuls + copies, then batched chain
    ddq8 = ap.tile([P, NT * M], F32, tag="ddq8", name="ddq8")
    dgq8 = ap.tile([P, NT], F32, tag="dgq8", name="dgq8")
    for t in range(NT):
        pt = psq.tile([P, M + P], F32, tag=f"pq{t % 2}", name="pq")
        nc.tensor.matmul(pt[:, 0:M], qhT[:, t * P:(t + 1) * P], wT_sb[:],
                         start=True, stop=True, skip_group_check=True)
        nc.tensor.matmul(pt[:, M:M + P], qhT[:, t * P:(t + 1) * P],
                         qhT[:, t * P:(t + 1) * P], start=True, stop=True,
                         skip_group_check=True)
        nc.scalar.activation(ddq8[:, t * M:(t + 1) * M], pt[:, 0:M], AF.Copy,
                             scale=1.0)
        scr = sp.tile([P, P], BF16, tag="scr128", name="scr128")
        nc.vector.scalar_tensor_tensor(
            scr[:], in0=pt[:, M:M + P], scalar=-0.5, in1=eye_f[:],
            op0=ALU.mult, op1=ALU.mult, accum_out=dgq8[:, t:t + 1])
    negmx8 = ap.tile([P, NT], F32, tag="negmx8", name="negmx8")
    nc.vector.tensor_reduce(negmx8[:],
                            ddq8[:].rearrange("p (t m) -> p t m", t=NT),
                            AX.X, ALU.max, negate=True)
    biasq8 = ap.tile([P, NT], F32, tag="biasq8", name="biasq8")
    nc.vector.tensor_tensor(biasq8[:], dgq8[:], negmx8[:], ALU.add)
    qp_raw8 = ap.tile([P, NT * M], F32, tag="qp_raw8", name="qp_raw8")
    den8c = ap.tile([P, NT], F32, tag="den8c", name="den8c")
    for t in range(NT):
        nc.scalar.activation(qp_raw8[:, t * M:(t + 1) * M],
                             ddq8[:, t * M:(t + 1) * M], AF.Exp,
                             bias=biasq8[:, t:t + 1], scale=1.0)
        scr64 = sp.tile([P, M], BF16, tag="scr64", name="scr64")
        nc.vector.scalar_tensor_tensor(
            scr64[:], in0=qp_raw8[:, t * M:(t + 1) * M], scalar=8.0,
            in1=ksum_b[:], op0=ALU.mult, op1=ALU.mult,
            accum_out=den8c[:, t:t + 1])
    nc.vector.tensor_scalar(out=den8c[:], in0=den8c[:], scalar1=seps8[:],
                            scalar2=None, op0=ALU.add)
    dinv8c = ap.tile([P, NT], F32, tag="dinv8c", name="dinv8c")
    nc.vector.reciprocal(dinv8c[:], den8c[:])
    edinv8c = ap.tile([P, NT], F32, tag="edinv8c", name="edinv8c")
    nc.vector.tensor_scalar(out=edinv8c[:], in0=dinv8c[:], scalar1=EPS_K,
                            scalar2=None, op0=ALU.mult)
    for t in range(NT):
        qp_hat = sp.tile([P, M], BF16, tag="qp_hat", name="qp_hat")
        nc.scalar.activation(qp_hat[:], qp_raw8[:, t * M:(t + 1) * M],
                             AF.Identity, bias=edinv8c[:, t:t + 1],
                             scale=dinv8c[:, t:t + 1])
        tps = psm.tile([M, P], BF16, tag="sps", name="sps")
        nc.tensor.transpose(tps[:], qp_hat[:], eye_bf[:])
        qpT_sb = sp.tile([M, P], BF16, tag="qpT_sb", name="qpT_sb")
        nc.scalar.activation(qpT_sb[:], tps[:], AF.Copy, scale=1.0)
        yps = psm.tile([P, P], F32, tag="sps", name="sps")
        nc.tensor.matmul(yps[:], ctx_sb[:], qpT_sb[:], start=True,
                         stop=True)
        nc.scalar.activation(yT8[:, hh * N + t * P: hh * N + (t + 1) * P],
                             yps[:], AF.Identity, scale=SY)
    if hh == 0 and "d_yt0" in dbg:
        nc.sync.dma_start(outs["d_yt0"], yT8[:, 0:N])


def _emit_mlp(tc, nc, ctx, ins, outs, dbg, flags, r1, h2T, h2T_cm,
              b1_pt, b2_b, sp):
    # fc1 + gelu -> a1T feature-major (48 x (P, N) bf16)
    a1p_cm = tc.tile_pool(name="a1p", bufs=1, side="right")
    a1p = a1p_cm.__enter__()
    a1T = [a1p.tile([P, N], BF16, tag=f"a1T{o}", name=f"a1T{o}") for o in range(OT1)]
    with tc.tile_pool(name="w1p", bufs=4) as w1p, \
         tc.tile_pool(name="psF1", bufs=3, space="PSUM") as psF1:
        for o in range(OT1):
            wt = w1p.tile([P, DC * P], BF16, tag="w1t", name="w1t")
            eng = nc.gpsimd if o % 2 == 0 else nc.sync
            eng.dma_start(wt[:], ins["w1r"][o])
            for hf in range(2):
                pst = psF1.tile([P, 512], F32, tag="psF1", name="psF1")
                for d in range(DC):
                    nc.tensor.matmul(
                        pst[:], wt[:, d * P:(d + 1) * P],
                        h2T[d][:, hf * 512:(hf + 1) * 512],
                        start=(d == 0), stop=(d == DC - 1))
                nc.scalar.activation(
                    a1T[o][:, hf * 512:(hf + 1) * 512], pst[:], AF.Gelu,
                    bias=b1_pt[:, o:o + 1], scale=1.0)
    h2T_cm.__exit__(None, None, None)
    if "d_a10" in dbg:
        nc.sync.dma_start(outs["d_a10"], a1T[0][:])

    # fc2: token-major out; rotated accumulation frees early weights for the
    # next j-block's prefetch
    with tc.tile_pool(name="w2p", bufs=1) as w2p, \
         tc.tile_pool(name="psF2", bufs=3, space="PSUM") as psF2:
        for j in range(3):
            w2 = []
            for oc in range(OT1):
                wt = w2p.tile([P, 512], BF16, tag=f"w2_{oc}", name=f"w2_{oc}")
                eng = nc.sync if oc % 2 == 0 else nc.gpsimd
                eng.dma_start(wt[:], ins["w2b"][j, oc])
                w2.append(wt)
            for t in range(NT):
                pst = psF2.tile([P, 512], F32, tag="psF2", name="psF2")
                rot = (t * 6) % OT1
                order = list(range(rot, OT1)) + list(range(0, rot))
                for i, oc in enumerate(order):
                    nc.tensor.matmul(
                        pst[:], a1T[oc][:, t * P:(t + 1) * P], w2[oc][:],
                        start=(i == 0), stop=(i == OT1 - 1))
                sl = r1[t][:, j * 512:(j + 1) * 512]
                nc.vector.tensor_tensor(sl, sl, pst[:], ALU.add)
                if j == 2:
                    if b2_b is not None:
                        nc.vector.tensor_tensor(r1[t][:], r1[t][:], b2_b[:],
                                                ALU.add)
                    nc.sync.dma_start(outs["out"][t * P:(t + 1) * P, :],
                                      r1[t][:])
    a1p_cm.__exit__(None, None, None)
